# revision 1
# baseline (speedup 1.0000x reference)
"""DIN attention kernel for Trainium2 (8 NeuronCores, data-parallel over batch).

Reference computation per sample b (B=2048, L=200, D=128, H1=256, H2=128):
  att_in = [q, k, q-k, q*k]            [L, 4D]
  h1 = prelu(att_in @ W1 + b1, a1)     [L, 256]
  h2 = prelu(h1 @ W2 + b2, a2)         [L, 128]
  s  = (h2 @ W3 + b3)[:, 0]            [L]
  p  = renorm(softmax(mask(s)) * mask) [L]
  out = p @ k                          [D]

Device algorithm (per core, 256 samples, processed in 4 blocks of 64 = 32 pairs):
  - W1 is split on host: W1a(q-part)+W1c, W1b-W1c(k-part), W1d(qk-part), so
    att_in @ W1 == q@(W1a+W1c) + k@(W1b-W1c) + (q*k)@W1d.
  - keys are loaded naturally [L, D], transposed on the PE to kT [D, L] (2 samples
    packed side by side -> matmul free dim 400 >= 256, which makes float32r
    matmuls run at 1 cycle/row).
  - h1^T, h2^T computed transposed [feat, L]; scores via per-sample matmuls with
    W3 as the moving operand producing score *columns* [L, 1] collected per
    64-sample block, transposed once per block for a batched masked softmax,
    transposed back for the weighted sum (keys natural layout as stationary).
  - b3 is dropped: softmax is shift-invariant.
"""

import sys

sys.path.insert(0, "/opt/trn_rl_repo")

import numpy as np

import concourse.bass as bass
import concourse.bacc as bacc
import concourse.mybir as mybir
import concourse.tile as tile
from concourse import masks
from concourse.bass_utils import run_bass_kernel_spmd

F32 = mybir.dt.float32
F32R = mybir.dt.float32r
F16 = mybir.dt.float16
ALU = mybir.AluOpType
ACTF = mybir.ActivationFunctionType
AX = mybir.AxisListType

B, L, D = 2048, 200, 128
H1, H2 = 256, 128
NCORES = 8
BC = B // NCORES  # samples per core

L0, L1R = 128, 72  # L split into 128 + 72 rows

_CACHE = {}


def _mm(nc, out, lhsT, rhs, start, stop, mm_dtype=None):
    nc.tensor.matmul(out, lhsT, rhs, start=start, stop=stop)


def _build(bc, blk, mode, a1v, a2v, mm_dtype=F16, tr_dtype=F32, reps=1):
    """Build the per-core Bass program. bc = samples/core, blk = block size."""
    assert bc % blk == 0 and blk % 2 == 0
    nblk = bc // blk
    npair_blk = blk // 2

    c3_1 = float((1.0 - a1v) / (1.0 + a1v))
    c3_2 = float((1.0 - a2v) / (1.0 + a2v))

    nc = bacc.Bacc("TRN2", target_bir_lowering=False, debug=False, num_devices=NCORES)

    def din(name, shape, dt=F32):
        return nc.dram_tensor(name, shape, dt, kind="ExternalInput").ap()

    keys_d = din("keys", [bc, L, D])
    qT_d = din("qT", [D, bc])
    maskf_d = din("maskf", [bc, L])
    w1ac_d = din("w1ac", [D, H1], F16)
    w1bc_d = din("w1bc", [D, H1], F16)
    w1d_d = din("w1d", [D, H1], F16)
    w2a_d = din("w2a", [128, H2], F16)
    w2b_d = din("w2b", [128, H2], F16)
    w3_d = din("w3", [H2, 1])
    b1r_d = din("b1r", [1, H1], F16)
    b2r_d = din("b2r", [1, H2], F16)
    b1c_d = din("b1c", [128, 2])
    b2c_d = din("b2c", [128, 1])
    acol_d = din("acol", [128, 2])  # [:,0]=a1, [:,1]=a2 broadcast columns
    out_d = nc.dram_tensor("out", [bc, D], F32, kind="ExternalOutput").ap()

    with tile.TileContext(nc) as tc:
        with (
            tc.tile_pool(name="const", bufs=1) as cpool,
            tc.tile_pool(name="kret", bufs=2) as kret,
            tc.tile_pool(name="work", bufs=2) as work,
            tc.tile_pool(name="blkw", bufs=2) as blkw,
            tc.tile_pool(name="ps_kt", bufs=1, space="PSUM") as ps_kt,
            tc.tile_pool(name="ps_h1", bufs=2, space="PSUM") as ps_h1,
            tc.tile_pool(name="ps_h2", bufs=1, space="PSUM") as ps_h2,
            tc.tile_pool(name="ps_sc", bufs=1, space="PSUM") as ps_sc,
            tc.tile_pool(name="ps_sm", bufs=1, space="PSUM") as ps_sm,
        ):
            ident = cpool.tile([128, 128], F32)
            masks.make_identity(nc, ident[:])
            identr = ident[:]

            w1ac = cpool.tile([D, H1], F16)
            w1bc = cpool.tile([D, H1], F16)
            w1d = cpool.tile([D, H1], F16)
            w2a = cpool.tile([128, H2], F16)
            w2b = cpool.tile([128, H2], F16)
            w3 = cpool.tile([H2, 1], F32)
            b1r = cpool.tile([1, H1], F16)
            b2r = cpool.tile([1, H2], F16)
            b1c = cpool.tile([128, 2], F32)
            b2c = cpool.tile([128, 1], F32)
            acol = cpool.tile([128, 2], F32)
            ones = cpool.tile([1, 400], F16)
            qT = cpool.tile([D, bc], F32)
            qT16 = cpool.tile([D, bc], F16)
            nc.sync.dma_start(w1ac[:], w1ac_d[:])
            nc.sync.dma_start(w1bc[:], w1bc_d[:])
            nc.sync.dma_start(w1d[:], w1d_d[:])
            nc.sync.dma_start(w2a[:], w2a_d[:])
            nc.sync.dma_start(w2b[:], w2b_d[:])
            nc.sync.dma_start(w3[:], w3_d[:])
            nc.sync.dma_start(b1r[:], b1r_d[:])
            nc.sync.dma_start(b2r[:], b2r_d[:])
            nc.sync.dma_start(b1c[:], b1c_d[:])
            nc.sync.dma_start(b2c[:], b2c_d[:])
            nc.sync.dma_start(acol[:], acol_d[:])
            nc.sync.dma_start(qT[:], qT_d[:])
            nc.vector.memset(ones[:], 1.0)
            nc.vector.tensor_copy(qT16[:], qT[:])

            for _rep in range(reps):
              for ib in range(nblk):
                s_base = ib * blk

                maskf = blkw.tile([blk, L], F32, tag="maskf")
                nc.sync.dma_start(maskf[:], maskf_d[s_base : s_base + blk, :])

                # long-lived per-block psum (one bank):
                #   cols 0:blk   = score cols l 0:128
                #   cols blk:2b  = score cols l 128:200 (partitions 0:72)
                #   cols 2b:3b   = u cols [d, blk]
                scps = ps_sc.tile([128, 3 * blk], F32, tag="scps")
                # softmax/transposed region (one bank)
                smps = ps_sm.tile([128, 456], F32, tag="smps")

                ka_tiles = []
                kb_tiles = []
                for p in range(npair_blk):
                    s0 = s_base + 2 * p
                    ka = kret.tile([128, 256], F32, tag=f"ka{p}")
                    kb = kret.tile([L1R, 256], F32, tag=f"kb{p}")
                    ka_tiles.append(ka)
                    kb_tiles.append(kb)
                    nc.sync.dma_start(ka[:, 0:128], keys_d[s0, 0:L0, :])
                    nc.sync.dma_start(ka[:, 128:256], keys_d[s0 + 1, 0:L0, :])
                    nc.sync.dma_start(kb[:, 0:128], keys_d[s0, L0:L, :])
                    nc.sync.dma_start(kb[:, 128:256], keys_d[s0 + 1, L0:L, :])

                    # kT for the pair: [128(d), 400(l s0 | l s1)]
                    ktp = ps_kt.tile([128, 400], F32, tag="ktp")
                    nc.tensor.matmul(
                        ktp[:, 0:128],
                        ka[:, 0:128], identr,
                        is_transpose=True)
                    nc.tensor.matmul(
                        ktp[:, 128:200],
                        kb[0:L1R, 0:128],
                        identr[0:L1R, 0:L1R], is_transpose=True)
                    nc.tensor.matmul(
                        ktp[:, 200:328],
                        ka[:, 128:256], identr,
                        is_transpose=True)
                    nc.tensor.matmul(
                        ktp[:, 328:400],
                        kb[0:L1R, 128:256],
                        identr[0:L1R, 0:L1R], is_transpose=True)

                    kt = work.tile([128, 400], F16, tag="kt")
                    nc.vector.tensor_copy(kt[:], ktp[:])

                    # p = q * k (columns of qT broadcast per sample)
                    pt = work.tile([128, 400], F16, tag="pt")
                    nc.vector.tensor_scalar_mul(
                        pt[:, 0:200], kt[:, 0:200], qT[:, s0 : s0 + 1])
                    nc.vector.tensor_scalar_mul(
                        pt[:, 200:400], kt[:, 200:400], qT[:, s0 + 1 : s0 + 2])

                    # layer 1 -> h1^T psum [128, 1024]: j0 at 0:400, j1 at 512:912
                    h1p = ps_h1.tile([128, 1024], F32, tag="h1p")
                    qbc = (
                        qT16[:, s0 : s0 + 2].unsqueeze(2).broadcast_to([D, 2, 200])
                    )
                    for jc in range(2):
                        cb = jc * 512
                        js = slice(jc * 128, (jc + 1) * 128)
                        o = h1p[:, cb : cb + 400]
                        _mm(nc, o, w1bc[:, js], kt[:], True, False, mm_dtype)
                        _mm(nc, o, w1d[:, js], pt[:], False, False, mm_dtype)
                        _mm(nc, o, w1ac[:, js], qbc, False, mode != "abs", mm_dtype)
                        if mode == "abs":
                            _mm(nc, o, b1r[0:1, js], ones[:], False, True, mm_dtype)

                    h1span = h1p[:].rearrange("p (c x) -> p c x", c=2)[:, :, 0:400]
                    h1s = work.tile([128, 800], F16, tag="h1s")
                    h1s3 = h1s[:].rearrange("p (c x) -> p c x", c=2)
                    if mode == "abs":
                        r1 = work.tile([128, 800], F16, tag="r1")
                        r13 = r1[:].rearrange("p (c x) -> p c x", c=2)
                        nc.scalar.activation(r13, h1span, ACTF.Abs, scale=c3_1)
                        nc.vector.tensor_tensor(h1s3, r13, h1span, op=ALU.add)
                    else:
                        for jc in range(2):
                            nc.scalar.activation(
                                h1s[:, jc * 400 : jc * 400 + 400],
                                h1p[:, jc * 512 : jc * 512 + 400],
                                ACTF.Prelu,
                                bias=b1c[:, jc : jc + 1],
                                scale=1.0,
                                alpha=float(a1v),
                            )

                    # layer 2 -> h2^T psum [128, 400]
                    h2p = ps_h2.tile([128, 400], F32, tag="h2p")
                    _mm(nc, h2p[:], w2a[:], h1s[:, 0:400], True, False, mm_dtype)
                    _mm(nc, h2p[:], w2b[:], h1s[:, 400:800], False, mode != "abs",
                        mm_dtype)
                    if mode == "abs":
                        _mm(nc, h2p[:], b2r[:], ones[:], False, True, mm_dtype)

                    h2s = work.tile([128, 400], F32, tag="h2s")
                    if mode == "abs":
                        r2 = work.tile([128, 400], F32, tag="r2")
                        nc.scalar.activation(r2[:], h2p[:], ACTF.Abs, scale=c3_2)
                        nc.vector.tensor_tensor(h2s[:], r2[:], h2p[:], op=ALU.add)
                    else:
                        nc.scalar.activation(
                            h2s[:], h2p[:], ACTF.Prelu,
                            bias=b2c[:],
                            scale=1.0, alpha=float(a2v))

                    # layer 3: score columns into block psum
                    for si in range(2):
                        bl = 2 * p + si
                        c0 = si * 200
                        _mm(nc, scps[0:128, bl : bl + 1],
                            h2s[:, c0 : c0 + 128], w3[:], True, True, mm_dtype)
                        _mm(nc, scps[0:L1R, blk + bl : blk + bl + 1],
                            h2s[:, c0 + 128 : c0 + 200], w3[:], True, True, mm_dtype)

                # ---- block tail: batched masked softmax over [blk, 200] ----
                sc0 = blkw.tile([128, blk], F32, tag="sc0")
                sc1 = blkw.tile([L1R, blk], F32, tag="sc1")
                nc.vector.tensor_copy(sc0[:], scps[0:128, 0:blk])
                nc.vector.tensor_copy(sc1[:], scps[0:L1R, blk : 2 * blk])
                # scores [blk, 200] = T(sc0)|T(sc1)
                scores = smps[0:blk, 0:200]
                nc.tensor.matmul(scores[:, 0:128],
                                 sc0[:],
                                 identr, is_transpose=True)
                nc.tensor.matmul(scores[:, 128:200],
                                 sc1[:],
                                 identr[0:L1R, 0:L1R], is_transpose=True)

                nm = blkw.tile([blk, 1], F32, tag="nm")
                nc.vector.tensor_reduce(
                    nm[:], scores, axis=AX.X, op=ALU.max, negate=True)
                e = blkw.tile([blk, L], F32, tag="e")
                nc.scalar.activation(e[:], scores, ACTF.Exp, bias=nm[:], scale=1.0)
                e2 = blkw.tile([blk, L], F32, tag="e2")
                den = blkw.tile([blk, 1], F32, tag="den")
                nc.vector.scalar_tensor_tensor(
                    e2[:], e[:], 1.0, maskf[:], op0=ALU.mult, op1=ALU.mult,
                    accum_out=den[:])
                rec = blkw.tile([blk, 1], F32, tag="rec")
                nc.vector.tensor_scalar_max(den[:], den[:], 1e-12)
                nc.vector.reciprocal(rec[:], den[:])
                probs = blkw.tile([blk, L], F32, tag="probs")
                nc.vector.tensor_scalar_mul(probs[:], e2[:], rec[:])

                # transpose probs back to columns
                pT0 = smps[0:128, 200:200 + blk]
                pT1 = smps[0:L1R, 200 + blk:200 + blk + blk]
                nc.tensor.matmul(pT0,
                                 probs[:, 0:128],
                                 identr[0:blk, 0:blk], is_transpose=True)
                nc.tensor.matmul(pT1,
                                 probs[:, 128:200],
                                 identr[0:blk, 0:blk], is_transpose=True)
                pT0s = blkw.tile([128, blk], F32, tag="pT0s")
                pT1s = blkw.tile([L1R, blk], F32, tag="pT1s")
                nc.vector.tensor_copy(pT0s[:], pT0)
                nc.vector.tensor_copy(pT1s[:], pT1)

                # weighted sum: u columns [128(d), blk]
                ucols = scps[0:128, 2 * blk : 3 * blk]
                for p in range(npair_blk):
                    ka, kb = ka_tiles[p], kb_tiles[p]
                    for si in range(2):
                        bl = 2 * p + si
                        ks = slice(si * 128, si * 128 + 128)
                        _mm(nc, ucols[:, bl : bl + 1], ka[:, ks],
                            pT0s[:, bl : bl + 1], True, False, mm_dtype)
                        _mm(nc, ucols[:, bl : bl + 1], kb[0:L1R, ks],
                            pT1s[0:L1R, bl : bl + 1], False, True, mm_dtype)

                usb = blkw.tile([128, blk], F32, tag="usb")
                nc.vector.tensor_copy(usb[:], ucols)
                uT = smps[0:blk, 328:456]
                nc.tensor.matmul(uT,
                                 usb[:], identr,
                                 is_transpose=True)
                osb = blkw.tile([blk, D], F32, tag="osb")
                nc.vector.tensor_copy(osb[:], uT)
                nc.sync.dma_start(out_d[s_base : s_base + blk, :], osb[:])

    nc.compile()
    return nc


def _prep_inputs(query, keys, mask, W1, b1, a1, W2, b2, a2, W3, mode):
    a1v, a2v = float(a1[0]), float(a2[0])
    W1 = np.asarray(W1, np.float32)
    w1a, w1b, w1c, w1d = W1[0:128], W1[128:256], W1[256:384], W1[384:512]
    w1ac = np.ascontiguousarray((w1a + w1c).astype(np.float16))
    w1bc = np.ascontiguousarray((w1b - w1c).astype(np.float16))
    w1d = np.ascontiguousarray(w1d.astype(np.float16))
    W2 = np.asarray(W2, np.float32)
    W3 = np.asarray(W3, np.float32)
    if mode == "abs":
        c1_1 = (1.0 + a1v) / 2.0
        c1_2 = (1.0 + a2v) / 2.0
        W2s = W2 * c1_1
        W3s = W3 * c1_2
    else:
        W2s, W3s = W2, W3
    w2a = np.ascontiguousarray(W2s[0:128].astype(np.float16))
    w2b = np.ascontiguousarray(W2s[128:256].astype(np.float16))
    b1r = np.asarray(b1, np.float16).reshape(1, H1)
    b2r = np.asarray(b2, np.float16).reshape(1, H2)
    b1cc = np.ascontiguousarray(np.asarray(b1, np.float32).reshape(2, 128).T)
    b2cc = np.ascontiguousarray(np.asarray(b2, np.float32).reshape(128, 1))
    acol = np.empty((128, 2), np.float32)
    acol[:, 0] = a1v
    acol[:, 1] = a2v

    shared = dict(w1ac=w1ac, w1bc=w1bc, w1d=w1d, w2a=w2a, w2b=w2b,
                  w3=W3s, b1r=b1r, b2r=b2r, b1c=b1cc, b2c=b2cc, acol=acol)

    bc = query.shape[0] // NCORES
    in_maps = []
    for c in range(NCORES):
        s = slice(c * bc, (c + 1) * bc)
        m = dict(shared)
        m["keys"] = np.ascontiguousarray(keys[s], np.float32)
        m["qT"] = np.ascontiguousarray(np.asarray(query[s], np.float32).T)
        m["maskf"] = np.ascontiguousarray(mask[s].astype(np.float32))
        in_maps.append(m)
    return in_maps, a1v, a2v


def kernel(query, keys, mask, W1, b1, a1, W2, b2, a2, W3, b3, mode="abs",
           blk=64, trace=False):
    query = np.asarray(query, np.float32)
    keys = np.asarray(keys, np.float32)
    bc = query.shape[0] // NCORES
    in_maps, a1v, a2v = _prep_inputs(
        query, keys, mask, W1, b1, a1, W2, b2, a2, W3, mode)

    key = (bc, blk, mode, a1v, a2v)
    if key not in _CACHE:
        _CACHE[key] = _build(bc, blk, mode, a1v, a2v)
    nc = _CACHE[key]

    res = run_bass_kernel_spmd(
        nc, in_maps, core_ids=list(range(NCORES)), trace=trace)
    out = np.concatenate([res.results[c]["out"] for c in range(NCORES)], axis=0)
    kernel.last_results = res
    return out.astype(np.float32)



# revision 3
# speedup vs baseline: 4.4310x; 4.4310x over previous
"""DIN attention kernel for Trainium2 (8 NeuronCores, data-parallel over batch).

Reference computation per sample b (B=2048, L=200, D=128, H1=256, H2=128):
  att_in = [q, k, q-k, q*k]            [L, 4D]
  h1 = prelu(att_in @ W1 + b1, a1)     [L, 256]
  h2 = prelu(h1 @ W2 + b2, a2)         [L, 128]
  s  = (h2 @ W3 + b3)[:, 0]            [L]
  p  = renorm(softmax(mask(s)) * mask) [L]
  out = p @ k                          [D]

Device algorithm (per core, 256 samples, processed in 4 blocks of 64 = 32 pairs):
  - W1 is split on host: W1a(q-part)+W1c, W1b-W1c(k-part), W1d(qk-part), so
    att_in @ W1 == q@(W1a+W1c) + k@(W1b-W1c) + (q*k)@W1d.
  - keys are loaded naturally [L, D], transposed on the PE to kT [D, L] (2 samples
    packed side by side -> matmul free dim 400 >= 256, which makes float32r
    matmuls run at 1 cycle/row).
  - h1^T, h2^T computed transposed [feat, L]; scores via per-sample matmuls with
    W3 as the moving operand producing score *columns* [L, 1] collected per
    64-sample block, transposed once per block for a batched masked softmax,
    transposed back for the weighted sum (keys natural layout as stationary).
  - b3 is dropped: softmax is shift-invariant.
"""

import sys

sys.path.insert(0, "/opt/trn_rl_repo")

import numpy as np

import concourse.bass as bass
import concourse.bacc as bacc
import concourse.mybir as mybir
import concourse.tile as tile
from concourse import masks
from concourse.bass_utils import run_bass_kernel_spmd

F32 = mybir.dt.float32
F32R = mybir.dt.float32r
F16 = mybir.dt.float16
ALU = mybir.AluOpType
ACTF = mybir.ActivationFunctionType
AX = mybir.AxisListType

B, L, D = 2048, 200, 128
H1, H2 = 256, 128
NCORES = 8
BC = B // NCORES  # samples per core

L0, L1R = 128, 72  # L split into 128 + 72 rows

_CACHE = {}


def _mm(nc, out, lhsT, rhs, start, stop, mm_dtype=None):
    nc.tensor.matmul(out, lhsT, rhs, start=start, stop=stop)


def _build(bc, blk, mode, a1v, a2v, mm_dtype=F16, tr_dtype=F32, reps=1):
    """Build the per-core Bass program. bc = samples/core, blk = block size."""
    assert bc % blk == 0 and blk % 2 == 0
    nblk = bc // blk
    npair_blk = blk // 2

    c3_1 = float((1.0 - a1v) / (1.0 + a1v))
    c3_2 = float((1.0 - a2v) / (1.0 + a2v))

    nc = bacc.Bacc("TRN2", target_bir_lowering=False, debug=False, num_devices=NCORES)

    def din(name, shape, dt=F32):
        return nc.dram_tensor(name, shape, dt, kind="ExternalInput").ap()

    keys_d = din("keys", [bc, L, D])
    qT_d = din("qT", [D, bc])
    maskf_d = din("maskf", [bc, L])
    w1ac_d = din("w1ac", [D, H1], F16)
    w1bc_d = din("w1bc", [D, H1], F16)
    w1d_d = din("w1d", [D, H1], F16)
    w2a_d = din("w2a", [128, H2], F16)
    w2b_d = din("w2b", [128, H2], F16)
    w3_d = din("w3", [H2, 1])
    b1r_d = din("b1r", [1, H1], F16)
    b2r_d = din("b2r", [1, H2], F16)
    b1c_d = din("b1c", [128, 2])
    b2c_d = din("b2c", [128, 1])
    acol_d = din("acol", [128, 2])  # [:,0]=a1, [:,1]=a2 broadcast columns
    out_d = nc.dram_tensor("out", [bc, D], F32, kind="ExternalOutput").ap()

    with tile.TileContext(nc) as tc:
        with (
            tc.tile_pool(name="const", bufs=1) as cpool,
            tc.tile_pool(name="kret", bufs=2) as kret,
            tc.tile_pool(name="work", bufs=2) as work,
            tc.tile_pool(name="blkw", bufs=2) as blkw,
            tc.tile_pool(name="ps_kt", bufs=1, space="PSUM") as ps_kt,
            tc.tile_pool(name="ps_h1", bufs=2, space="PSUM") as ps_h1,
            tc.tile_pool(name="ps_h2", bufs=1, space="PSUM") as ps_h2,
            tc.tile_pool(name="ps_sc", bufs=1, space="PSUM") as ps_sc,
            tc.tile_pool(name="ps_sm", bufs=1, space="PSUM") as ps_sm,
        ):
            ident = cpool.tile([128, 128], F32)
            masks.make_identity(nc, ident[:])
            identr = ident[:]

            w1ac = cpool.tile([D, H1], F16)
            w1bc = cpool.tile([D, H1], F16)
            w1d = cpool.tile([D, H1], F16)
            w2a = cpool.tile([128, H2], F16)
            w2b = cpool.tile([128, H2], F16)
            w3 = cpool.tile([H2, 1], F32)
            b1r = cpool.tile([1, H1], F16)
            b2r = cpool.tile([1, H2], F16)
            b1c = cpool.tile([128, 2], F32)
            b2c = cpool.tile([128, 1], F32)
            acol = cpool.tile([128, 2], F32)
            ones = cpool.tile([1, 400], F16)
            qT = cpool.tile([D, bc], F32)
            qT16 = cpool.tile([D, bc], F16)
            nc.sync.dma_start(w1ac[:], w1ac_d[:])
            nc.sync.dma_start(w1bc[:], w1bc_d[:])
            nc.sync.dma_start(w1d[:], w1d_d[:])
            nc.sync.dma_start(w2a[:], w2a_d[:])
            nc.sync.dma_start(w2b[:], w2b_d[:])
            nc.sync.dma_start(w3[:], w3_d[:])
            nc.sync.dma_start(b1r[:], b1r_d[:])
            nc.sync.dma_start(b2r[:], b2r_d[:])
            nc.sync.dma_start(b1c[:], b1c_d[:])
            nc.sync.dma_start(b2c[:], b2c_d[:])
            nc.sync.dma_start(acol[:], acol_d[:])
            nc.sync.dma_start(qT[:], qT_d[:])
            nc.vector.memset(ones[:], 1.0)
            nc.vector.tensor_copy(qT16[:], qT[:])

            for _rep in range(reps):
              for ib in range(nblk):
                s_base = ib * blk

                maskf = blkw.tile([blk, L], F32, tag="maskf")
                nc.sync.dma_start(maskf[:], maskf_d[s_base : s_base + blk, :])

                # long-lived per-block psum (one bank):
                #   cols 0:blk   = score cols l 0:128
                #   cols blk:2b  = score cols l 128:200 (partitions 0:72)
                #   cols 2b:3b   = u cols [d, blk]
                scps = ps_sc.tile([128, 3 * blk], F32, tag="scps")
                # softmax/transposed region (one bank)
                smps = ps_sm.tile([128, 456], F32, tag="smps")

                ka_tiles = []
                kb_tiles = []
                for p in range(npair_blk):
                    s0 = s_base + 2 * p
                    ka = kret.tile([128, 256], F32, tag=f"ka{p}")
                    kb = kret.tile([L1R, 256], F32, tag=f"kb{p}")
                    ka_tiles.append(ka)
                    kb_tiles.append(kb)
                    nc.sync.dma_start(ka[:, 0:128], keys_d[s0, 0:L0, :])
                    nc.sync.dma_start(ka[:, 128:256], keys_d[s0 + 1, 0:L0, :])
                    nc.sync.dma_start(kb[:, 0:128], keys_d[s0, L0:L, :])
                    nc.sync.dma_start(kb[:, 128:256], keys_d[s0 + 1, L0:L, :])

                    # kT for the pair: [128(d), 400(l s0 | l s1)]
                    ktp = ps_kt.tile([128, 400], F32, tag="ktp")
                    nc.tensor.matmul(
                        ktp[:, 0:128],
                        ka[:, 0:128], identr,
                        is_transpose=True)
                    nc.tensor.matmul(
                        ktp[:, 128:200],
                        kb[0:L1R, 0:128],
                        identr[0:L1R, 0:L1R], is_transpose=True)
                    nc.tensor.matmul(
                        ktp[:, 200:328],
                        ka[:, 128:256], identr,
                        is_transpose=True)
                    nc.tensor.matmul(
                        ktp[:, 328:400],
                        kb[0:L1R, 128:256],
                        identr[0:L1R, 0:L1R], is_transpose=True)

                    kt = work.tile([128, 400], F16, tag="kt")
                    nc.vector.tensor_copy(kt[:], ktp[:])

                    # p = q * k (columns of qT broadcast per sample)
                    pt = work.tile([128, 400], F16, tag="pt")
                    nc.vector.tensor_scalar_mul(
                        pt[:, 0:200], kt[:, 0:200], qT[:, s0 : s0 + 1])
                    nc.vector.tensor_scalar_mul(
                        pt[:, 200:400], kt[:, 200:400], qT[:, s0 + 1 : s0 + 2])

                    # layer 1 -> h1^T psum [128, 1024]: j0 at 0:400, j1 at 512:912
                    h1p = ps_h1.tile([128, 1024], F32, tag="h1p")
                    qbc = (
                        qT16[:, s0 : s0 + 2].unsqueeze(2).broadcast_to([D, 2, 200])
                    )
                    for jc in range(2):
                        cb = jc * 512
                        js = slice(jc * 128, (jc + 1) * 128)
                        o = h1p[:, cb : cb + 400]
                        _mm(nc, o, w1bc[:, js], kt[:], True, False, mm_dtype)
                        _mm(nc, o, w1d[:, js], pt[:], False, False, mm_dtype)
                        _mm(nc, o, w1ac[:, js], qbc, False, mode != "abs", mm_dtype)
                        if mode == "abs":
                            _mm(nc, o, b1r[0:1, js], ones[:], False, True, mm_dtype)

                    h1span = h1p[:].rearrange("p (c x) -> p c x", c=2)[:, :, 0:400]
                    h1s = work.tile([128, 800], F16, tag="h1s")
                    h1s3 = h1s[:].rearrange("p (c x) -> p c x", c=2)
                    if mode == "abs":
                        r1 = work.tile([128, 800], F16, tag="r1")
                        r13 = r1[:].rearrange("p (c x) -> p c x", c=2)
                        nc.scalar.activation(r13, h1span, ACTF.Abs, scale=c3_1)
                        nc.vector.tensor_tensor(h1s3, r13, h1span, op=ALU.add)
                    else:
                        for jc in range(2):
                            nc.scalar.activation(
                                h1s[:, jc * 400 : jc * 400 + 400],
                                h1p[:, jc * 512 : jc * 512 + 400],
                                ACTF.Prelu,
                                bias=b1c[:, jc : jc + 1],
                                scale=1.0,
                                alpha=float(a1v),
                            )

                    # layer 2 -> h2^T psum [128, 400]
                    h2p = ps_h2.tile([128, 400], F32, tag="h2p")
                    _mm(nc, h2p[:], w2a[:], h1s[:, 0:400], True, False, mm_dtype)
                    _mm(nc, h2p[:], w2b[:], h1s[:, 400:800], False, mode != "abs",
                        mm_dtype)
                    if mode == "abs":
                        _mm(nc, h2p[:], b2r[:], ones[:], False, True, mm_dtype)

                    h2s = work.tile([128, 400], F32, tag="h2s")
                    if mode == "abs":
                        r2 = work.tile([128, 400], F32, tag="r2")
                        nc.scalar.activation(r2[:], h2p[:], ACTF.Abs, scale=c3_2)
                        nc.vector.tensor_tensor(h2s[:], r2[:], h2p[:], op=ALU.add)
                    else:
                        nc.scalar.activation(
                            h2s[:], h2p[:], ACTF.Prelu,
                            bias=b2c[:],
                            scale=1.0, alpha=float(a2v))

                    # layer 3: score columns into block psum
                    for si in range(2):
                        bl = 2 * p + si
                        c0 = si * 200
                        _mm(nc, scps[0:128, bl : bl + 1],
                            h2s[:, c0 : c0 + 128], w3[:], True, True, mm_dtype)
                        _mm(nc, scps[0:L1R, blk + bl : blk + bl + 1],
                            h2s[:, c0 + 128 : c0 + 200], w3[:], True, True, mm_dtype)

                # ---- block tail: batched masked softmax over [blk, 200] ----
                sc0 = blkw.tile([128, blk], F32, tag="sc0")
                sc1 = blkw.tile([L1R, blk], F32, tag="sc1")
                nc.vector.tensor_copy(sc0[:], scps[0:128, 0:blk])
                nc.vector.tensor_copy(sc1[:], scps[0:L1R, blk : 2 * blk])
                # scores [blk, 200] = T(sc0)|T(sc1)
                scores = smps[0:blk, 0:200]
                nc.tensor.matmul(scores[:, 0:128],
                                 sc0[:],
                                 identr, is_transpose=True)
                nc.tensor.matmul(scores[:, 128:200],
                                 sc1[:],
                                 identr[0:L1R, 0:L1R], is_transpose=True)

                nm = blkw.tile([blk, 1], F32, tag="nm")
                nc.vector.tensor_reduce(
                    nm[:], scores, axis=AX.X, op=ALU.max, negate=True)
                e = blkw.tile([blk, L], F32, tag="e")
                nc.scalar.activation(e[:], scores, ACTF.Exp, bias=nm[:], scale=1.0)
                e2 = blkw.tile([blk, L], F32, tag="e2")
                den = blkw.tile([blk, 1], F32, tag="den")
                nc.vector.scalar_tensor_tensor(
                    e2[:], e[:], 1.0, maskf[:], op0=ALU.mult, op1=ALU.mult,
                    accum_out=den[:])
                rec = blkw.tile([blk, 1], F32, tag="rec")
                nc.vector.tensor_scalar_max(den[:], den[:], 1e-12)
                nc.vector.reciprocal(rec[:], den[:])
                probs = blkw.tile([blk, L], F32, tag="probs")
                nc.vector.tensor_scalar_mul(probs[:], e2[:], rec[:])

                # transpose probs back to columns
                pT0 = smps[0:128, 200:200 + blk]
                pT1 = smps[0:L1R, 200 + blk:200 + blk + blk]
                nc.tensor.matmul(pT0,
                                 probs[:, 0:128],
                                 identr[0:blk, 0:blk], is_transpose=True)
                nc.tensor.matmul(pT1,
                                 probs[:, 128:200],
                                 identr[0:blk, 0:blk], is_transpose=True)
                pT0s = blkw.tile([128, blk], F32, tag="pT0s")
                pT1s = blkw.tile([L1R, blk], F32, tag="pT1s")
                nc.vector.tensor_copy(pT0s[:], pT0)
                nc.vector.tensor_copy(pT1s[:], pT1)

                # weighted sum: u columns [128(d), blk]
                ucols = scps[0:128, 2 * blk : 3 * blk]
                for p in range(npair_blk):
                    ka, kb = ka_tiles[p], kb_tiles[p]
                    for si in range(2):
                        bl = 2 * p + si
                        ks = slice(si * 128, si * 128 + 128)
                        _mm(nc, ucols[:, bl : bl + 1], ka[:, ks],
                            pT0s[:, bl : bl + 1], True, False, mm_dtype)
                        _mm(nc, ucols[:, bl : bl + 1], kb[0:L1R, ks],
                            pT1s[0:L1R, bl : bl + 1], False, True, mm_dtype)

                usb = blkw.tile([128, blk], F32, tag="usb")
                nc.vector.tensor_copy(usb[:], ucols)
                uT = smps[0:blk, 328:456]
                nc.tensor.matmul(uT,
                                 usb[:], identr,
                                 is_transpose=True)
                osb = blkw.tile([blk, D], F32, tag="osb")
                nc.vector.tensor_copy(osb[:], uT)
                nc.sync.dma_start(out_d[s_base : s_base + blk, :], osb[:])

    nc.compile()
    return nc


def _build_v3(bc, blk, a1v, a2v):
    """v3: host-transposed f16 key layouts, q-term folded into activation bias.

    Per pair (2 samples): 4 L1 matmuls (FD=400), 4 Prelu activations (scalar),
    2 L2 matmuls, 2-op DVE prelu, 4 tiny L3 matmuls; block tail does batched
    softmax + weighted sum (stationary = natural-layout keys from HBM).
    """
    assert bc % blk == 0 and blk % 2 == 0
    nblk = bc // blk
    npair_blk = blk // 2

    nc = bacc.Bacc("TRN2", target_bir_lowering=False, debug=False, num_devices=NCORES)

    def din(name, shape, dt=F32):
        return nc.dram_tensor(name, shape, dt, kind="ExternalInput").ap()

    ktr_d = din("ktr", [D, bc * L], F16)     # [d, (s, l)]
    knat_d = din("knat", [L, bc * D], F16)   # [l, (s, d)]
    qT_d = din("qT", [D, bc])
    qT16_d = din("qT16", [D, bc], F16)
    maskf_d = din("maskf", [bc, L])
    w1ac_d = din("w1ac", [D, H1], F16)
    w1bc_d = din("w1bc", [D, H1], F16)
    w1d_d = din("w1d", [D, H1], F16)
    w2a_d = din("w2a", [128, H2], F16)
    w2b_d = din("w2b", [128, H2], F16)
    w3_d = din("w3", [H2, 1], F16)
    b1c_d = din("b1c", [128, 2])
    b2c_d = din("b2c", [128, 1])
    out_d = nc.dram_tensor("out", [bc, D], F32, kind="ExternalOutput").ap()

    with tile.TileContext(nc) as tc:
        with (
            tc.tile_pool(name="const", bufs=1) as cpool,
            tc.tile_pool(name="kchunk", bufs=2) as kchunk,
            tc.tile_pool(name="work", bufs=2) as work,
            tc.tile_pool(name="blkw", bufs=2) as blkw,
            tc.tile_pool(name="ps_h1", bufs=2, space="PSUM") as ps_h1,
            tc.tile_pool(name="ps_h2", bufs=2, space="PSUM") as ps_h2,
            tc.tile_pool(name="ps_sc", bufs=1, space="PSUM") as ps_sc,
            tc.tile_pool(name="ps_sm", bufs=1, space="PSUM") as ps_sm,
        ):
            ident = cpool.tile([128, 128], F32)
            masks.make_identity(nc, ident[:])
            identr = ident[:]

            w1ac = cpool.tile([D, H1], F16)
            w1bc = cpool.tile([D, H1], F16)
            w1d = cpool.tile([D, H1], F16)
            w2a = cpool.tile([128, H2], F16)
            w2b = cpool.tile([128, H2], F16)
            w3 = cpool.tile([H2, 1], F16)
            b1c = cpool.tile([128, 2], F32)
            b2c = cpool.tile([128, 1], F32)
            qT = cpool.tile([D, bc], F32)
            qT16 = cpool.tile([D, bc], F16)
            nc.sync.dma_start(w1ac[:], w1ac_d[:])
            nc.sync.dma_start(w1bc[:], w1bc_d[:])
            nc.sync.dma_start(w1d[:], w1d_d[:])
            nc.sync.dma_start(w2a[:], w2a_d[:])
            nc.sync.dma_start(w2b[:], w2b_d[:])
            nc.sync.dma_start(w3[:], w3_d[:])
            nc.sync.dma_start(b1c[:], b1c_d[:])
            nc.sync.dma_start(b2c[:], b2c_d[:])
            nc.sync.dma_start(qT[:], qT_d[:])
            nc.sync.dma_start(qT16[:], qT16_d[:])

            # qb1[j, jc*bc + s] = (q @ (W1a + W1c))[s, jc*128 + j] + b1[jc*128 + j]
            qh1p = ps_sm.tile([128, 2 * bc], F32, tag="smps")
            nc.tensor.matmul(qh1p[:, 0:bc], w1ac[:, 0:128], qT16[:],
                             start=True, stop=True)
            nc.tensor.matmul(qh1p[:, bc : 2 * bc], w1ac[:, 128:256], qT16[:],
                             start=True, stop=True)
            qb1 = cpool.tile([128, 2 * bc], F32)
            for jc in range(2):
                nc.scalar.activation(
                    qb1[:, jc * bc : (jc + 1) * bc],
                    qh1p[:, jc * bc : (jc + 1) * bc],
                    ACTF.Identity, bias=b1c[:, jc : jc + 1], scale=1.0)

            for ib in range(nblk):
                s_base = ib * blk

                # block-chunk DMAs (contiguous, MB-scale)
                ktrch = kchunk.tile([D, blk * L], F16, tag="ktrch")
                kna = kchunk.tile([128, blk * D], F16, tag="kna")
                knb = kchunk.tile([L1R, blk * D], F16, tag="knb")
                nc.sync.dma_start(
                    ktrch[:], ktr_d[:, s_base * L : (s_base + blk) * L])
                nc.sync.dma_start(
                    kna[:], knat_d[0:L0, s_base * D : (s_base + blk) * D])
                nc.sync.dma_start(
                    knb[:], knat_d[L0:L, s_base * D : (s_base + blk) * D])
                maskf = blkw.tile([blk, L], F32, tag="maskf")
                nc.sync.dma_start(maskf[:], maskf_d[s_base : s_base + blk, :])

                # per-block psum: score cols [l0 0:blk | l1 blk:2b | u cols 2b:3b]
                scps = ps_sc.tile([128, 3 * blk], F32, tag="scps")
                smps = ps_sm.tile([128, 456], F32, tag="smps")

                for p in range(npair_blk):
                    s0 = s_base + 2 * p
                    kt = ktrch[:, (2 * p) * L : (2 * p) * L + 2 * L]

                    pt = work.tile([128, 2 * L], F16, tag="pt")
                    nc.vector.tensor_scalar_mul(
                        pt[:, 0:L], kt[:, 0:L], qT[:, s0 : s0 + 1])
                    nc.vector.tensor_scalar_mul(
                        pt[:, L : 2 * L], kt[:, L : 2 * L], qT[:, s0 + 1 : s0 + 2])

                    # L1 -> h1^T psum [128, 1024]: jc at jc*512, cols (si, l)
                    h1p = ps_h1.tile([128, 1024], F32, tag="h1p")
                    for jc in range(2):
                        js = slice(jc * 128, (jc + 1) * 128)
                        o = h1p[:, jc * 512 : jc * 512 + 2 * L]
                        nc.tensor.matmul(o, w1bc[:, js], kt, start=True, stop=False)
                        nc.tensor.matmul(o, w1d[:, js], pt[:], start=False, stop=True)

                    h1s = work.tile([128, 4 * L], F16, tag="h1s")
                    for jc in range(2):
                        for si in range(2):
                            nc.scalar.activation(
                                h1s[:, jc * 2 * L + si * L : jc * 2 * L + si * L + L],
                                h1p[:, jc * 512 + si * L : jc * 512 + si * L + L],
                                ACTF.Prelu,
                                bias=qb1[:, jc * bc + s0 + si : jc * bc + s0 + si + 1],
                                scale=1.0,
                                alpha=float(a1v))

                    # L2 -> h2^T psum [128, 400]
                    h2p = ps_h2.tile([128, 2 * L], F32, tag="h2p")
                    nc.tensor.matmul(h2p[:], w2a[:], h1s[:, 0 : 2 * L],
                                     start=True, stop=False)
                    nc.tensor.matmul(h2p[:], w2b[:], h1s[:, 2 * L : 4 * L],
                                     start=False, stop=True)

                    h2b = work.tile([128, 2 * L], F16, tag="h2b")
                    nc.vector.tensor_scalar_add(h2b[:], h2p[:], b2c[:, 0:1])
                    h2s = work.tile([128, 2 * L], F16, tag="h2s")
                    nc.vector.scalar_tensor_tensor(
                        h2s[:], h2b[:], float(a2v), h2b[:],
                        op0=ALU.mult, op1=ALU.max)

                    # L3: score columns into block psum
                    for si in range(2):
                        bl = 2 * p + si
                        c0 = si * L
                        nc.tensor.matmul(
                            scps[0:128, bl : bl + 1],
                            h2s[:, c0 : c0 + L0], w3[:], start=True, stop=True)
                        nc.tensor.matmul(
                            scps[0:L1R, blk + bl : blk + bl + 1],
                            h2s[:, c0 + L0 : c0 + L], w3[:], start=True, stop=True)

                # ---- block tail: batched masked softmax over [blk, 200] ----
                sc0 = blkw.tile([128, blk], F32, tag="sc0")
                sc1 = blkw.tile([L1R, blk], F32, tag="sc1")
                nc.vector.tensor_copy(sc0[:], scps[0:128, 0:blk])
                nc.vector.tensor_copy(sc1[:], scps[0:L1R, blk : 2 * blk])
                scores = smps[0:blk, 0:200]
                nc.tensor.matmul(scores[:, 0:128], sc0[:], identr,
                                 is_transpose=True)
                nc.tensor.matmul(scores[:, 128:200], sc1[:],
                                 identr[0:L1R, 0:L1R], is_transpose=True)

                nm = blkw.tile([blk, 1], F32, tag="nm")
                nc.vector.tensor_reduce(
                    nm[:], scores, axis=AX.X, op=ALU.max, negate=True)
                e = blkw.tile([blk, L], F32, tag="e")
                nc.scalar.activation(e[:], scores, ACTF.Exp, bias=nm[:], scale=1.0)
                e2 = blkw.tile([blk, L], F32, tag="e2")
                den = blkw.tile([blk, 1], F32, tag="den")
                nc.vector.scalar_tensor_tensor(
                    e2[:], e[:], 1.0, maskf[:], op0=ALU.mult, op1=ALU.mult,
                    accum_out=den[:])
                rec = blkw.tile([blk, 1], F32, tag="rec")
                nc.vector.tensor_scalar_max(den[:], den[:], 1e-12)
                nc.vector.reciprocal(rec[:], den[:])
                probs = blkw.tile([blk, L], F32, tag="probs")
                nc.vector.tensor_scalar_mul(probs[:], e2[:], rec[:])

                pT0 = smps[0:128, 200 : 200 + blk]
                pT1 = smps[0:L1R, 200 + blk : 200 + 2 * blk]
                nc.tensor.matmul(pT0, probs[:, 0:128],
                                 identr[0:blk, 0:blk], is_transpose=True)
                nc.tensor.matmul(pT1, probs[:, 128:200],
                                 identr[0:blk, 0:blk], is_transpose=True)
                pT0s = blkw.tile([128, blk], F32, tag="pT0s")
                pT1s = blkw.tile([L1R, blk], F32, tag="pT1s")
                nc.vector.tensor_copy(pT0s[:], pT0)
                nc.vector.tensor_copy(pT1s[:], pT1)
                pT0h = blkw.tile([128, blk], F16, tag="pT0h")
                pT1h = blkw.tile([L1R, blk], F16, tag="pT1h")
                nc.vector.tensor_copy(pT0h[:], pT0s[:])
                nc.vector.tensor_copy(pT1h[:], pT1s[:])

                # weighted sum: u columns [128(d), blk]
                ucols = scps[0:128, 2 * blk : 3 * blk]
                for bl in range(blk):
                    ks = slice(bl * D, (bl + 1) * D)
                    nc.tensor.matmul(ucols[:, bl : bl + 1], kna[:, ks],
                                     pT0h[:, bl : bl + 1], start=True, stop=False)
                    nc.tensor.matmul(ucols[:, bl : bl + 1], knb[0:L1R, ks],
                                     pT1h[0:L1R, bl : bl + 1],
                                     start=False, stop=True)

                usb = blkw.tile([128, blk], F32, tag="usb")
                nc.vector.tensor_copy(usb[:], ucols)
                uT = smps[0:blk, 328:456]
                nc.tensor.matmul(uT, usb[:], identr, is_transpose=True)
                osb = blkw.tile([blk, D], F32, tag="osb")
                nc.vector.tensor_copy(osb[:], uT)
                nc.sync.dma_start(out_d[s_base : s_base + blk, :], osb[:])

    nc.compile()
    return nc


def _prep_inputs_v3(query, keys, mask, W1, b1, a1, W2, b2, a2, W3):
    a1v, a2v = float(a1[0]), float(a2[0])
    W1 = np.asarray(W1, np.float32)
    w1a, w1b, w1c, w1d = W1[0:128], W1[128:256], W1[256:384], W1[384:512]
    w1ac = np.ascontiguousarray((w1a + w1c).astype(np.float16))
    w1bc = np.ascontiguousarray((w1b - w1c).astype(np.float16))
    w1d = np.ascontiguousarray(w1d.astype(np.float16))
    W2 = np.asarray(W2, np.float32)
    w2a = np.ascontiguousarray(W2[0:128].astype(np.float16))
    w2b = np.ascontiguousarray(W2[128:256].astype(np.float16))
    w3 = np.ascontiguousarray(np.asarray(W3, np.float16).reshape(H2, 1))
    b1cc = np.ascontiguousarray(np.asarray(b1, np.float32).reshape(2, 128).T)
    b2cc = np.ascontiguousarray(np.asarray(b2, np.float32).reshape(128, 1))

    shared = dict(w1ac=w1ac, w1bc=w1bc, w1d=w1d, w2a=w2a, w2b=w2b,
                  w3=w3, b1c=b1cc, b2c=b2cc)

    bc = query.shape[0] // NCORES
    keys16 = np.asarray(keys, np.float16)
    query32 = np.asarray(query, np.float32)
    in_maps = []
    for c in range(NCORES):
        s = slice(c * bc, (c + 1) * bc)
        kc = keys16[s]  # [bc, L, D]
        m = dict(shared)
        m["ktr"] = np.ascontiguousarray(
            kc.transpose(2, 0, 1)).reshape(D, bc * L)
        m["knat"] = np.ascontiguousarray(
            kc.transpose(1, 0, 2)).reshape(L, bc * D)
        qTc = np.ascontiguousarray(query32[s].T)
        m["qT"] = qTc
        m["qT16"] = np.ascontiguousarray(qTc.astype(np.float16))
        m["maskf"] = np.ascontiguousarray(mask[s].astype(np.float32))
        in_maps.append(m)
    return in_maps, a1v, a2v


def _prep_inputs(query, keys, mask, W1, b1, a1, W2, b2, a2, W3, mode):
    a1v, a2v = float(a1[0]), float(a2[0])
    W1 = np.asarray(W1, np.float32)
    w1a, w1b, w1c, w1d = W1[0:128], W1[128:256], W1[256:384], W1[384:512]
    w1ac = np.ascontiguousarray((w1a + w1c).astype(np.float16))
    w1bc = np.ascontiguousarray((w1b - w1c).astype(np.float16))
    w1d = np.ascontiguousarray(w1d.astype(np.float16))
    W2 = np.asarray(W2, np.float32)
    W3 = np.asarray(W3, np.float32)
    if mode == "abs":
        c1_1 = (1.0 + a1v) / 2.0
        c1_2 = (1.0 + a2v) / 2.0
        W2s = W2 * c1_1
        W3s = W3 * c1_2
    else:
        W2s, W3s = W2, W3
    w2a = np.ascontiguousarray(W2s[0:128].astype(np.float16))
    w2b = np.ascontiguousarray(W2s[128:256].astype(np.float16))
    b1r = np.asarray(b1, np.float16).reshape(1, H1)
    b2r = np.asarray(b2, np.float16).reshape(1, H2)
    b1cc = np.ascontiguousarray(np.asarray(b1, np.float32).reshape(2, 128).T)
    b2cc = np.ascontiguousarray(np.asarray(b2, np.float32).reshape(128, 1))
    acol = np.empty((128, 2), np.float32)
    acol[:, 0] = a1v
    acol[:, 1] = a2v

    shared = dict(w1ac=w1ac, w1bc=w1bc, w1d=w1d, w2a=w2a, w2b=w2b,
                  w3=W3s, b1r=b1r, b2r=b2r, b1c=b1cc, b2c=b2cc, acol=acol)

    bc = query.shape[0] // NCORES
    in_maps = []
    for c in range(NCORES):
        s = slice(c * bc, (c + 1) * bc)
        m = dict(shared)
        m["keys"] = np.ascontiguousarray(keys[s], np.float32)
        m["qT"] = np.ascontiguousarray(np.asarray(query[s], np.float32).T)
        m["maskf"] = np.ascontiguousarray(mask[s].astype(np.float32))
        in_maps.append(m)
    return in_maps, a1v, a2v


def kernel(query, keys, mask, W1, b1, a1, W2, b2, a2, W3, b3, mode="v3",
           blk=64, trace=False):
    query = np.asarray(query, np.float32)
    keys = np.asarray(keys, np.float32)
    bc = query.shape[0] // NCORES
    if mode == "v3":
        in_maps, a1v, a2v = _prep_inputs_v3(
            query, keys, mask, W1, b1, a1, W2, b2, a2, W3)
    else:
        in_maps, a1v, a2v = _prep_inputs(
            query, keys, mask, W1, b1, a1, W2, b2, a2, W3, mode)

    key = (bc, blk, mode, a1v, a2v)
    if key not in _CACHE:
        if mode == "v3":
            _CACHE[key] = _build_v3(bc, blk, a1v, a2v)
        else:
            _CACHE[key] = _build(bc, blk, mode, a1v, a2v)
    nc = _CACHE[key]

    res = run_bass_kernel_spmd(
        nc, in_maps, core_ids=list(range(NCORES)), trace=trace)
    out = np.concatenate([res.results[c]["out"] for c in range(NCORES)], axis=0)
    kernel.last_results = res
    return out.astype(np.float32)

